# revision 18
# baseline (speedup 1.0000x reference)
"""Trainium2 Bass kernel for nn_DSVF (differentiable SVF filter, forward).

The reference applies an SVF biquad via FFT overlap-add (rfft/irfft at
NFFT=4096 over 2048-sample segments).  Because the biquad's poles are
well damped (radius ~0.5 for any plausible parameter draw), the aliased
impulse response decays below 1e-40 within 128 taps, so the whole
operation is numerically identical to a plain causal FIR applied to
each batch row (zero initial condition).  This kernel covers taps
0..255 exactly (truncation error ~|pole|^256, i.e. zero in fp32).

Layout (chosen so ALL device work is dense bf16 matmul + copies):
data-parallel over batch rows, 8 rows per core.  Each 262144-sample row
is framed column-major: z[k, c] = x[c*128 + k] (host-side transpose),
so SBUF holds [128 fine-time partitions x 2048+1 cols] per row with one
zero column prepended (zero initial condition / "col -1").

The FIR then splits into exactly two dense matmuls per output bank:
  out[m, c] = sum_k W0[k, m] z[k, c]  +  sum_k W1[k, m] z[k, c-1]
with W0[k, m] = h[m-k] (taps 0..127, lower-triangular Toeplitz) and
W1[k, m] = h[128+m-k] (taps 1..255, full).  W0/W1 are the *stationary*
operands — loaded once per phase — and the signal streams through as
512-wide bf16 moving operands, accumulating in PSUM (fp32).  The spill
across columns is just "same stream shifted one column", so there is no
halo duplication and no on-device transpose.

bf16 on the wire halves HBM traffic (the roofline: ~4.46 MB in +
4.19 MB out per core at ~358 GB/s ≈ 24 us); fp32 accumulation in PSUM
plus fp64 host tap computation keeps rel-err ~1e-3 << 2e-2.
"""

import os
import sys

import numpy as np
import ml_dtypes
ml_dtypes.float16 = __import__("numpy").float16  # fp16: more mantissa than bf16, same wire cost

for _p in ("/opt/trn_rl_repo",):
    if _p not in sys.path:
        sys.path.insert(0, _p)

N_CORES = 8
BATCH = 64
L = 262144
ROWS = BATCH // N_CORES  # rows per core
P = 128  # partitions == fine-time frame == contraction dim
C = L // P  # 2048 columns per row
CP = C + 1  # +1 zero column at each row start
T = 256  # FIR taps covered exactly (0..255)
BANK = 512  # PSUM bank width in fp32 == matmul moving width
NB = C // BANK  # 4 banks per row

_built = None

# Profiling knobs (used by the local test harness, not by grading):
TRACE = False
TRACE_DIR = None
LAST_RESULTS = None


def _filter_taps(g, R, m_hp, m_bp, m_lp):
    """First T taps of the biquad impulse response, float64 recursion."""
    g = float(g)
    R = float(R)
    gt = np.tan(np.pi * (1.0 / (1.0 + np.exp(-g))) / 2.0)
    Rt = np.log1p(np.exp(R))
    g2 = gt * gt
    b = (
        g2 * m_lp + gt * m_bp + m_hp,
        2 * g2 * m_lp - 2 * m_hp,
        g2 * m_lp - gt * m_bp + m_hp,
    )
    a = (g2 + 2 * Rt * gt + 1, 2 * g2 - 2, g2 - 2 * Rt * gt + 1)
    h = np.zeros(T, dtype=np.float64)
    for n in range(T):
        acc = b[n] if n < 3 else 0.0
        if n >= 1:
            acc -= a[1] * h[n - 1]
        if n >= 2:
            acc -= a[2] * h[n - 2]
        h[n] = acc / a[0]
    return h


def _weights(h):
    """[P, 2P] bf16: cols [0,P) = W0 (taps m-k), cols [P,2P) = W1 (128+m-k)."""
    k = np.arange(P)[:, None]
    m = np.arange(P)[None, :]
    d0 = m - k
    w0 = np.where(d0 >= 0, h[np.clip(d0, 0, T - 1)], 0.0)
    w1 = h[128 + d0]  # 128+m-k in [1, 255] always
    return np.concatenate([w0, w1], axis=1).astype(ml_dtypes.float16)


def _host_layout(x_shard):
    """[ROWS, L] fp32 -> [P, ROWS*CP] bf16, col-major frames + zero col."""
    z = np.zeros((P, ROWS * CP), dtype=ml_dtypes.float16)
    zt = x_shard.reshape(ROWS, C, P).transpose(0, 2, 1)  # [r, k, c]
    for r in range(ROWS):
        z[:, r * CP + 1 : (r + 1) * CP] = zt[r].astype(ml_dtypes.float16)
    return z


def _host_unlayout(y_core):
    """[P, ROWS*C] bf16 -> [ROWS, L] fp32."""
    return (
        y_core.reshape(P, ROWS, C)
        .transpose(1, 2, 0)
        .reshape(ROWS, L)
        .astype(np.float32)
    )


def _build():
    global _built
    if _built is not None:
        return _built

    from contextlib import ExitStack

    import concourse.bacc as bacc
    import concourse.mybir as mybir
    from concourse import tile

    f32 = mybir.dt.float32
    bf16 = mybir.dt.float16

    nc = bacc.Bacc("TRN2", target_bir_lowering=False, debug=False)

    XZ = nc.dram_tensor("xz", [P, ROWS * CP], bf16, kind="ExternalInput").ap()
    W = nc.dram_tensor("w", [P, 2 * P], bf16, kind="ExternalInput").ap()
    Y = nc.dram_tensor("y", [P, ROWS * C], bf16, kind="ExternalOutput").ap()

    with tile.TileContext(nc) as tc, ExitStack() as ctx:
        const_pool = ctx.enter_context(tc.tile_pool(name="const", bufs=1))
        x_pool = ctx.enter_context(tc.tile_pool(name="x", bufs=1))
        y_pool = ctx.enter_context(tc.tile_pool(name="y", bufs=1))
        po_pool = ctx.enter_context(tc.tile_pool(name="po", bufs=4, space="PSUM"))

        w_sb = const_pool.tile([P, 2 * P], bf16)

        xz_sb = x_pool.tile([P, ROWS * CP], bf16)
        y_sb = y_pool.tile([P, ROWS * C], bf16)

        # input DMAs: issue split across both HWDGE rings (sync/scalar) so
        # descriptor generation (~0.6us each) runs in parallel and rows
        # arrive in consumption order (even rows on the scalar ring, odd on
        # sync).  ALL output DMAs go on the sync ring, *behind* its input
        # rows in the queue FIFO: inputs drain at full rate first (they
        # gate the PE), and the output backlog then drains at full rate
        # instead of trickling at evac pace.
        nc.scalar.dma_start(xz_sb[:, 0 : 2 * BANK + 1], XZ[:, 0 : 2 * BANK + 1])
        nc.sync.dma_start(w_sb[:], W[:])
        nc.scalar.dma_start(xz_sb[:, 2 * BANK + 1 : CP], XZ[:, 2 * BANK + 1 : CP])
        H1 = 2 * BANK + 1  # half-row split point (first half includes zero col)
        nc.sync.dma_start(xz_sb[:, CP : CP + H1], XZ[:, CP : CP + H1])
        nc.sync.dma_start(xz_sb[:, CP + H1 : 2 * CP], XZ[:, CP + H1 : 2 * CP])
        nc.scalar.dma_start(xz_sb[:, 2 * CP : 2 * CP + H1], XZ[:, 2 * CP : 2 * CP + H1])
        nc.scalar.dma_start(xz_sb[:, 2 * CP + H1 : 3 * CP], XZ[:, 2 * CP + H1 : 3 * CP])
        nc.sync.dma_start(xz_sb[:, 3 * CP : 3 * CP + H1], XZ[:, 3 * CP : 3 * CP + H1])
        nc.sync.dma_start(xz_sb[:, 3 * CP + H1 : 4 * CP], XZ[:, 3 * CP + H1 : 4 * CP])
        nc.scalar.dma_start(xz_sb[:, 4 * CP : 6 * CP], XZ[:, 4 * CP : 6 * CP])
        nc.scalar.dma_start(xz_sb[:, 6 * CP : 8 * CP], XZ[:, 6 * CP : 8 * CP])

        # PE warmup on the (tiny, early) weight tile: ~2us of matmul busy
        # while row 0 streams in keeps the HAM activity window filling, so
        # real matmuls reach 2.4 GHz quickly.
        po_w = po_pool.tile([P, 2 * BANK], f32, name="powarm", tag="po")
        for i in range(4):
            nc.tensor.matmul(
                po_w[:, 0:256],
                w_sb[:, 0:P],
                w_sb[:, 0:256],
                start=(i == 0),
                stop=(i == 3),
            )

        for r in range(ROWS):
            last = r == ROWS - 1
            for h in range(2):  # half-rows: 2-bank PSUM tiles, bufs=4
                po = po_pool.tile([P, 2 * BANK], f32, name=f"po{r}_{h}", tag="po")
                for b in range(2):  # W0: in-column taps 0..127
                    col = h * 2 * BANK + b * BANK
                    nc.tensor.matmul(
                        po[:, b * BANK : (b + 1) * BANK],
                        w_sb[:, 0:P],
                        xz_sb[:, r * CP + 1 + col : r * CP + 1 + col + BANK],
                        start=True,
                        stop=False,
                    )
                for b in range(2):  # W1: spill taps 1..255 from prev column
                    col = h * 2 * BANK + b * BANK
                    nc.tensor.matmul(
                        po[:, b * BANK : (b + 1) * BANK],
                        w_sb[:, P : 2 * P],
                        xz_sb[:, r * CP + col : r * CP + col + BANK],
                        start=False,
                        stop=True,
                    )
                # evacuate this half-row in one 1024-wide copy (fewer
                # instructions => fewer event sems to clear in the epilogue);
                # DVE and ACT alternate by half-row.
                base = r * C + h * 2 * BANK
                if h == 0:
                    nc.vector.tensor_copy(y_sb[:, base : base + 2 * BANK], po[:, 0 : 2 * BANK])
                else:
                    nc.scalar.copy(y_sb[:, base : base + 2 * BANK], po[:, 0 : 2 * BANK])
                if last:  # fine-grained final stores to shorten the tail
                    nc.sync.dma_start(
                        Y[:, base : base + 2 * BANK], y_sb[:, base : base + 2 * BANK]
                    )
            if r in (1, 3, 5):  # pair stores, behind the inputs in sync's FIFO
                nc.sync.dma_start(
                    Y[:, (r - 1) * C : (r + 1) * C], y_sb[:, (r - 1) * C : (r + 1) * C]
                )
            elif r == 6:
                nc.sync.dma_start(
                    Y[:, r * C : (r + 1) * C], y_sb[:, r * C : (r + 1) * C]
                )

    nc.compile()
    _built = nc
    return nc


def kernel(x, g, R, m_hp, m_bp, m_lp):
    x = np.ascontiguousarray(np.asarray(x, dtype=np.float32))
    h = _filter_taps(
        np.asarray(g).reshape(-1)[0],
        np.asarray(R).reshape(-1)[0],
        float(np.asarray(m_hp).reshape(-1)[0]),
        float(np.asarray(m_bp).reshape(-1)[0]),
        float(np.asarray(m_lp).reshape(-1)[0]),
    )
    w = _weights(h)

    nc = _build()
    from concourse.bass_utils import run_bass_kernel_spmd

    in_maps = [
        {"xz": _host_layout(x[c * ROWS : (c + 1) * ROWS]), "w": w}
        for c in range(N_CORES)
    ]
    global LAST_RESULTS
    kwargs = {}
    if TRACE:
        kwargs = {"trace": True, "tmpdir": TRACE_DIR}
    res = run_bass_kernel_spmd(nc, in_maps, list(range(N_CORES)), **kwargs)
    LAST_RESULTS = res
    y = np.concatenate(
        [_host_unlayout(res.results[c]["y"]) for c in range(N_CORES)], axis=0
    )
    return y.astype(np.float32, copy=False)


# revision 19
# speedup vs baseline: 1.0201x; 1.0201x over previous
"""Trainium2 Bass kernel for nn_DSVF (differentiable SVF filter, forward).

The reference applies an SVF biquad via FFT overlap-add (rfft/irfft at
NFFT=4096 over 2048-sample segments).  Because the biquad's poles are
well damped (radius ~0.5 for any plausible parameter draw), the aliased
impulse response decays below 1e-40 within 128 taps, so the whole
operation is numerically identical to a plain causal FIR applied to
each batch row (zero initial condition).  This kernel covers taps
0..255 exactly (truncation error ~|pole|^256, i.e. zero in fp32).

Layout (chosen so ALL device work is dense bf16 matmul + copies):
data-parallel over batch rows, 8 rows per core.  Each 262144-sample row
is framed column-major: z[k, c] = x[c*128 + k] (host-side transpose),
so SBUF holds [128 fine-time partitions x 2048+1 cols] per row with one
zero column prepended (zero initial condition / "col -1").

The FIR then splits into exactly two dense matmuls per output bank:
  out[m, c] = sum_k W0[k, m] z[k, c]  +  sum_k W1[k, m] z[k, c-1]
with W0[k, m] = h[m-k] (taps 0..127, lower-triangular Toeplitz) and
W1[k, m] = h[128+m-k] (taps 1..255, full).  W0/W1 are the *stationary*
operands — loaded once per phase — and the signal streams through as
512-wide bf16 moving operands, accumulating in PSUM (fp32).  The spill
across columns is just "same stream shifted one column", so there is no
halo duplication and no on-device transpose.

bf16 on the wire halves HBM traffic (the roofline: ~4.46 MB in +
4.19 MB out per core at ~358 GB/s ≈ 24 us); fp32 accumulation in PSUM
plus fp64 host tap computation keeps rel-err ~1e-3 << 2e-2.
"""

import os
import sys

import numpy as np
import ml_dtypes
ml_dtypes.float16 = __import__("numpy").float16  # fp16: more mantissa than bf16, same wire cost

for _p in ("/opt/trn_rl_repo",):
    if _p not in sys.path:
        sys.path.insert(0, _p)

N_CORES = 8
BATCH = 64
L = 262144
ROWS = BATCH // N_CORES  # rows per core
P = 128  # partitions == fine-time frame == contraction dim
C = L // P  # 2048 columns per row
CP = C + 1  # +1 zero column at each row start
T = 256  # FIR taps covered exactly (0..255)
BANK = 512  # PSUM bank width in fp32 == matmul moving width
NB = C // BANK  # 4 banks per row

_built = None

# Profiling knobs (used by the local test harness, not by grading):
TRACE = False
TRACE_DIR = None
LAST_RESULTS = None


def _filter_taps(g, R, m_hp, m_bp, m_lp):
    """First T taps of the biquad impulse response, float64 recursion."""
    g = float(g)
    R = float(R)
    gt = np.tan(np.pi * (1.0 / (1.0 + np.exp(-g))) / 2.0)
    Rt = np.log1p(np.exp(R))
    g2 = gt * gt
    b = (
        g2 * m_lp + gt * m_bp + m_hp,
        2 * g2 * m_lp - 2 * m_hp,
        g2 * m_lp - gt * m_bp + m_hp,
    )
    a = (g2 + 2 * Rt * gt + 1, 2 * g2 - 2, g2 - 2 * Rt * gt + 1)
    h = np.zeros(T, dtype=np.float64)
    for n in range(T):
        acc = b[n] if n < 3 else 0.0
        if n >= 1:
            acc -= a[1] * h[n - 1]
        if n >= 2:
            acc -= a[2] * h[n - 2]
        h[n] = acc / a[0]
    return h


def _weights(h):
    """[P, 2P] bf16: cols [0,P) = W0 (taps m-k), cols [P,2P) = W1 (128+m-k)."""
    k = np.arange(P)[:, None]
    m = np.arange(P)[None, :]
    d0 = m - k
    w0 = np.where(d0 >= 0, h[np.clip(d0, 0, T - 1)], 0.0)
    w1 = h[128 + d0]  # 128+m-k in [1, 255] always
    return np.concatenate([w0, w1], axis=1).astype(ml_dtypes.float16)


def _host_layout(x_shard):
    """[ROWS, L] fp32 -> [P, ROWS*CP] bf16, col-major frames + zero col."""
    z = np.zeros((P, ROWS * CP), dtype=ml_dtypes.float16)
    zt = x_shard.reshape(ROWS, C, P).transpose(0, 2, 1)  # [r, k, c]
    for r in range(ROWS):
        z[:, r * CP + 1 : (r + 1) * CP] = zt[r].astype(ml_dtypes.float16)
    return z


def _host_unlayout(y_core):
    """[P, ROWS*C] bf16 -> [ROWS, L] fp32."""
    return (
        y_core.reshape(P, ROWS, C)
        .transpose(1, 2, 0)
        .reshape(ROWS, L)
        .astype(np.float32)
    )


def _build():
    global _built
    if _built is not None:
        return _built

    from contextlib import ExitStack

    import concourse.bacc as bacc
    import concourse.mybir as mybir
    from concourse import tile

    f32 = mybir.dt.float32
    bf16 = mybir.dt.float16

    nc = bacc.Bacc("TRN2", target_bir_lowering=False, debug=False)

    XZ = nc.dram_tensor("xz", [P, ROWS * CP], bf16, kind="ExternalInput").ap()
    W = nc.dram_tensor("w", [P, 2 * P], bf16, kind="ExternalInput").ap()
    Y = nc.dram_tensor("y", [P, ROWS * C], bf16, kind="ExternalOutput").ap()

    with tile.TileContext(nc) as tc, ExitStack() as ctx:
        const_pool = ctx.enter_context(tc.tile_pool(name="const", bufs=1))
        x_pool = ctx.enter_context(tc.tile_pool(name="x", bufs=1))
        y_pool = ctx.enter_context(tc.tile_pool(name="y", bufs=1))
        po_pool = ctx.enter_context(tc.tile_pool(name="po", bufs=4, space="PSUM"))

        w_sb = const_pool.tile([P, 2 * P], bf16)

        xz_sb = x_pool.tile([P, ROWS * CP], bf16)
        y_sb = y_pool.tile([P, ROWS * C], bf16)

        # input DMAs: issue split across both HWDGE rings (sync/scalar) so
        # descriptor generation (~0.6us each) runs in parallel and rows
        # arrive in consumption order (even rows on the scalar ring, odd on
        # sync).  ALL output DMAs go on the sync ring, *behind* its input
        # rows in the queue FIFO: inputs drain at full rate first (they
        # gate the PE), and the output backlog then drains at full rate
        # instead of trickling at evac pace.
        nc.scalar.dma_start(xz_sb[:, 0 : 2 * BANK + 1], XZ[:, 0 : 2 * BANK + 1])
        nc.sync.dma_start(w_sb[:], W[:])
        nc.scalar.dma_start(xz_sb[:, 2 * BANK + 1 : CP], XZ[:, 2 * BANK + 1 : CP])
        H1 = 2 * BANK + 1  # half-row split point (first half includes zero col)
        nc.sync.dma_start(xz_sb[:, CP : CP + H1], XZ[:, CP : CP + H1])
        nc.sync.dma_start(xz_sb[:, CP + H1 : 2 * CP], XZ[:, CP + H1 : 2 * CP])
        nc.scalar.dma_start(xz_sb[:, 2 * CP : 2 * CP + H1], XZ[:, 2 * CP : 2 * CP + H1])
        nc.scalar.dma_start(xz_sb[:, 2 * CP + H1 : 3 * CP], XZ[:, 2 * CP + H1 : 3 * CP])
        nc.sync.dma_start(xz_sb[:, 3 * CP : 3 * CP + H1], XZ[:, 3 * CP : 3 * CP + H1])
        nc.sync.dma_start(xz_sb[:, 3 * CP + H1 : 4 * CP], XZ[:, 3 * CP + H1 : 4 * CP])
        nc.scalar.dma_start(xz_sb[:, 4 * CP : 6 * CP], XZ[:, 4 * CP : 6 * CP])
        nc.sync.dma_start(xz_sb[:, 6 * CP : 8 * CP], XZ[:, 6 * CP : 8 * CP])

        # PE warmup on the (tiny, early) weight tile: ~2us of matmul busy
        # while row 0 streams in keeps the HAM activity window filling, so
        # real matmuls reach 2.4 GHz quickly.
        po_w = po_pool.tile([P, 2 * BANK], f32, name="powarm", tag="po")
        for i in range(4):
            nc.tensor.matmul(
                po_w[:, 0:256],
                w_sb[:, 0:P],
                w_sb[:, 0:256],
                start=(i == 0),
                stop=(i == 3),
            )

        for r in range(ROWS):
            last = r == ROWS - 1
            for h in range(2):  # half-rows: 2-bank PSUM tiles, bufs=4
                po = po_pool.tile([P, 2 * BANK], f32, name=f"po{r}_{h}", tag="po")
                for b in range(2):  # W0: in-column taps 0..127
                    col = h * 2 * BANK + b * BANK
                    nc.tensor.matmul(
                        po[:, b * BANK : (b + 1) * BANK],
                        w_sb[:, 0:P],
                        xz_sb[:, r * CP + 1 + col : r * CP + 1 + col + BANK],
                        start=True,
                        stop=False,
                    )
                for b in range(2):  # W1: spill taps 1..255 from prev column
                    col = h * 2 * BANK + b * BANK
                    nc.tensor.matmul(
                        po[:, b * BANK : (b + 1) * BANK],
                        w_sb[:, P : 2 * P],
                        xz_sb[:, r * CP + col : r * CP + col + BANK],
                        start=False,
                        stop=True,
                    )
                # evacuate this half-row in one 1024-wide copy (fewer
                # instructions => fewer event sems to clear in the epilogue);
                # DVE and ACT alternate by half-row.
                base = r * C + h * 2 * BANK
                if h == 0:
                    nc.vector.tensor_copy(y_sb[:, base : base + 2 * BANK], po[:, 0 : 2 * BANK])
                else:
                    nc.scalar.copy(y_sb[:, base : base + 2 * BANK], po[:, 0 : 2 * BANK])
                if last:  # fine-grained final stores to shorten the tail
                    nc.sync.dma_start(
                        Y[:, base : base + 2 * BANK], y_sb[:, base : base + 2 * BANK]
                    )
            if r in (1, 3, 5):  # pair stores, behind the inputs in sync's FIFO
                nc.sync.dma_start(
                    Y[:, (r - 1) * C : (r + 1) * C], y_sb[:, (r - 1) * C : (r + 1) * C]
                )
            elif r == 6:
                nc.sync.dma_start(
                    Y[:, r * C : (r + 1) * C], y_sb[:, r * C : (r + 1) * C]
                )

    nc.compile()
    _built = nc
    return nc


def kernel(x, g, R, m_hp, m_bp, m_lp):
    x = np.ascontiguousarray(np.asarray(x, dtype=np.float32))
    h = _filter_taps(
        np.asarray(g).reshape(-1)[0],
        np.asarray(R).reshape(-1)[0],
        float(np.asarray(m_hp).reshape(-1)[0]),
        float(np.asarray(m_bp).reshape(-1)[0]),
        float(np.asarray(m_lp).reshape(-1)[0]),
    )
    w = _weights(h)

    nc = _build()
    from concourse.bass_utils import run_bass_kernel_spmd

    in_maps = [
        {"xz": _host_layout(x[c * ROWS : (c + 1) * ROWS]), "w": w}
        for c in range(N_CORES)
    ]
    global LAST_RESULTS
    kwargs = {}
    if TRACE:
        kwargs = {"trace": True, "tmpdir": TRACE_DIR}
    res = run_bass_kernel_spmd(nc, in_maps, list(range(N_CORES)), **kwargs)
    LAST_RESULTS = res
    y = np.concatenate(
        [_host_unlayout(res.results[c]["y"]) for c in range(N_CORES)], axis=0
    )
    return y.astype(np.float32, copy=False)
